# revision 1
# baseline (speedup 1.0000x reference)
"""LinearRNNBlock Trainium2 kernel.

B=8, T=2048, C=1024, EXP=4. Data-parallel over batch: core b computes batch b.

On-chip layout is feature-major [c partitions, t free] end to end:
  - host pre-transposes x[b] -> [C, T] and pre-tiles all weights into
    [K=128, M=128] lhsT blocks, so the device does zero transposes
  - rmsnorm reductions (over C = partitions) go through a ones-matmul on the
    PE whose M=128 output is the per-token sum already broadcast across all
    partitions; rsqrt is exp(-0.5*ln(ms+eps)) on ScalarE
  - cumsum over T is a DVE tensor_tensor_scan along the free dim (fp32 state)
  - matmuls run with fp32 data bitcast to float32r (full PE rate at N=512)
"""

import os
import sys

sys.path.insert(0, "/opt/trn_rl_repo")

from contextlib import ExitStack

import numpy as np

import concourse.bass as bass
import concourse.tile as tile
from concourse import bacc
from concourse import mybir
from concourse.bass import ts
from concourse.bass_utils import run_bass_kernel_spmd

P = 128
B = 8
T = 2048
C = 1024
E = 4096
CK = C // P   # 8 channel chunks
EK = E // P   # 32 expanded chunks
TB = 512      # t-block (max fp32 matmul free dim / one PSUM bank)
NB = T // TB  # 4 t-blocks
EPS = 1e-6

F32 = mybir.dt.float32
F32R = mybir.dt.float32r
AF = mybir.ActivationFunctionType
OP = mybir.AluOpType

N_CORES = 8

_CACHED = {}


def _r(ap):
    return ap.bitcast(F32R)


def _build_program(reps=1):
    nc = bacc.Bacc("TRN2", target_bir_lowering=False, debug=False,
                   enable_asserts=False, num_devices=N_CORES)

    xt = nc.dram_tensor("xt", [CK, P, T], F32, kind="ExternalInput").ap()
    w1 = nc.dram_tensor("w1", [CK, P, CK * P], F32R, kind="ExternalInput").ap()
    b1 = nc.dram_tensor("b1", [P, CK], F32, kind="ExternalInput").ap()
    w21 = nc.dram_tensor("w21", [EK, P, CK * P], F32R, kind="ExternalInput").ap()
    b21 = nc.dram_tensor("b21", [P, EK], F32, kind="ExternalInput").ap()
    w22 = nc.dram_tensor("w22", [CK, 4, P, 8 * P], F32R, kind="ExternalInput").ap()
    b22 = nc.dram_tensor("b22", [P, CK], F32, kind="ExternalInput").ap()
    sci = nc.dram_tensor("sci", [P, T], F32, kind="ExternalInput").ap()
    onesd = nc.dram_tensor("onesd", [P, P], F32R, kind="ExternalInput").ap()
    yt = nc.dram_tensor("yt", [CK, P, T], F32, kind="ExternalOutput").ap()

    with tile.TileContext(nc) as tc, ExitStack() as ctx:
        consts = ctx.enter_context(tc.tile_pool(name="consts", bufs=1))
        arena = ctx.enter_context(tc.tile_pool(name="arena", bufs=1))
        wp = ctx.enter_context(tc.tile_pool(name="wp", bufs=5))
        bc = ctx.enter_context(tc.tile_pool(name="bc", bufs=2))
        h2p = ctx.enter_context(tc.tile_pool(name="h2p", bufs=8))
        gp = ctx.enter_context(tc.tile_pool(name="gp", bufs=2))
        sqp = ctx.enter_context(tc.tile_pool(name="sqp", bufs=4))
        ps = ctx.enter_context(tc.tile_pool(name="ps", bufs=1, space="PSUM"))

        ones = consts.tile([P, P], F32R)
        nc.sync.dma_start(out=ones, in_=onesd)
        epsb = consts.tile([P, 1], F32)
        nc.vector.memset(epsb, EPS)
        b1s = consts.tile([P, CK], F32)
        nc.sync.dma_start(out=b1s, in_=b1)
        b21s = consts.tile([P, EK], F32)
        nc.sync.dma_start(out=b21s, in_=b21)
        b22s = consts.tile([P, CK], F32)
        nc.sync.dma_start(out=b22s, in_=b22)
        scib = bc.tile([P, T], F32, tag="bcast", name="scib")
        nc.sync.dma_start(out=scib, in_=sci)

        # xa: x^T chunks -> out1 (in place) -> y (in place)
        xa = arena.tile([P, CK, T], F32)
        # sa: h1 -> state (in place); later reused as the relu activations `a`
        sa = arena.tile([P, CK, T], F32R)
        # view of sa as [P, EK, TB]: per-t-block storage of a = relu(...)
        af = sa.rearrange("p c t -> p (c t)").rearrange("p (e n) -> p e n", e=EK)

        from contextlib import nullcontext
        loop_ctx = tc.For_i(0, reps, 1) if reps > 1 else nullcontext()
        with loop_ctx:
            _emit_body(nc, tc, locals())

    nc.compile()
    return nc


def _emit_body(nc, tc, env):
    xa, sa, af = env["xa"], env["sa"], env["af"]
    ones, epsb, scib = env["ones"], env["epsb"], env["scib"]
    b1s, b21s, b22s = env["b1s"], env["b21s"], env["b22s"]
    xt, w1, w21, w22, yt = env["xt"], env["w1"], env["w21"], env["w22"], env["yt"]
    wp, bc, h2p, gp, sqp, ps = (env["wp"], env["bc"], env["h2p"], env["gp"],
                                env["sqp"], env["ps"])
    if True:
        for cc in range(CK):
            nc.sync.dma_start(out=xa[:, cc, :], in_=xt[cc])

        # ---- norm1: rstd1[t] broadcast across partitions ----
        rstd1 = bc.tile([P, T], F32, tag="bcast", name="rstd1")
        for tb in range(NB):
            acc = ps.tile([P, TB], F32, tag="acc", bufs=2, name=f"acc1_{tb}")
            for cc in range(CK):
                sq = sqp.tile([P, TB], F32R, tag="sq", name="sq1")
                nc.vector.tensor_mul(sq, xa[:, cc, ts(tb, TB)], xa[:, cc, ts(tb, TB)])
                nc.tensor.matmul(acc, lhsT=ones, rhs=sq,
                                 start=(cc == 0), stop=(cc == CK - 1))
            ln1 = gp.tile([P, TB], F32, tag="g", name="ln1")
            nc.scalar.activation(ln1, acc, AF.Ln, bias=epsb, scale=1.0 / C)
            nc.scalar.activation(rstd1[:, ts(tb, TB)], ln1, AF.Exp, scale=-0.5)

        # ---- h1 = x*rstd1 ; state = cumsum_t(h1) * scaler_recip ----
        # sliced per t-block with chained scan initial so mlp1 tb=0 can
        # start before later t-blocks are scanned
        scst = []
        for tb in range(NB):
            for cc in range(CK):
                s_sl = sa[:, cc, ts(tb, TB)]
                nc.vector.tensor_mul(s_sl, xa[:, cc, ts(tb, TB)], rstd1[:, ts(tb, TB)])
                init = 0.0 if tb == 0 else scst[(tb - 1) * CK + cc]
                nc.vector.tensor_tensor_scan(s_sl, s_sl, s_sl,
                                             initial=init, op0=OP.add, op1=OP.bypass)
                carry = gp.tile([P, 1], F32, tag="carry", bufs=CK + 2, name="carry")
                nc.vector.tensor_copy(carry, s_sl[:, TB - 1:TB])
                scst.append(carry)
                nc.vector.tensor_mul(s_sl, s_sl, scib[:, ts(tb, TB)])

        # ---- mlp1: gate = sigmoid(state @ W1 + b1); out1 = gate * x (in place in xa)
        for dc in range(CK):
            w1s = wp.tile([P, CK, P], F32R, tag="wk", name="w1s")
            nc.sync.dma_start(out=w1s, in_=w1[dc])
            for tb in range(NB):
                pg = ps.tile([P, TB], F32, tag="mm", bufs=4, name="pg")
                for cc in range(CK):
                    nc.tensor.matmul(pg, lhsT=w1s[:, cc, :],
                                     rhs=sa[:, cc, ts(tb, TB)],
                                     start=(cc == 0), stop=(cc == CK - 1))
                g = gp.tile([P, TB], F32, tag="g", name="g")
                nc.scalar.activation(g, pg, AF.Sigmoid, bias=b1s[:, dc:dc + 1], scale=1.0)
                nc.vector.tensor_mul(xa[:, dc, ts(tb, TB)], g, xa[:, dc, ts(tb, TB)])

        # ---- norm2 on out1 ----
        rstd2 = bc.tile([P, T], F32, tag="bcast", name="rstd2")
        for tb in range(NB):
            acc2 = ps.tile([P, TB], F32, tag="acc", bufs=2, name=f"acc2_{tb}")
            for cc in range(CK):
                sq2 = sqp.tile([P, TB], F32R, tag="sq", name="sq2")
                nc.vector.tensor_mul(sq2, xa[:, cc, ts(tb, TB)], xa[:, cc, ts(tb, TB)])
                nc.tensor.matmul(acc2, lhsT=ones, rhs=sq2,
                                 start=(cc == 0), stop=(cc == CK - 1))
            ln2 = gp.tile([P, TB], F32, tag="g", name="ln2")
            nc.scalar.activation(ln2, acc2, AF.Ln, bias=epsb, scale=1.0 / C)
            nc.scalar.activation(rstd2[:, ts(tb, TB)], ln2, AF.Exp, scale=-0.5)

        # ---- mlp2 per t-block: a = relu(h2 @ W21 + b21); y = a @ W22 + b22 + out1
        for tb in range(NB):
            h2ts = []
            for cc in range(CK):
                h2t = h2p.tile([P, TB], F32R, tag="h2", name="h2t")
                nc.vector.tensor_mul(h2t, xa[:, cc, ts(tb, TB)], rstd2[:, ts(tb, TB)])
                h2ts.append(h2t)
            for ec in range(EK):
                w21s = wp.tile([P, CK, P], F32R, tag="wk", name="w21s")
                nc.sync.dma_start(out=w21s, in_=w21[ec])
                pa = ps.tile([P, TB], F32, tag="mm", bufs=4, name="pa")
                for cc in range(CK):
                    nc.tensor.matmul(pa, lhsT=w21s[:, cc, :], rhs=h2ts[cc],
                                     start=(cc == 0), stop=(cc == CK - 1))
                nc.vector.tensor_scalar(out=af[:, ec, :], in0=pa,
                                        scalar1=b21s[:, ec:ec + 1], scalar2=0.0,
                                        op0=OP.add, op1=OP.max)
            for dc in range(CK):
                py = ps.tile([P, TB], F32, tag="py", bufs=2, name="py")
                for q in range(4):
                    w22s = wp.tile([P, 8, P], F32R, tag="wk", name="w22s")
                    nc.sync.dma_start(out=w22s, in_=w22[dc, q])
                    for j in range(8):
                        nc.tensor.matmul(py, lhsT=w22s[:, j, :],
                                         rhs=af[:, q * 8 + j, :],
                                         start=(q == 0 and j == 0),
                                         stop=(q == 3 and j == 7))
                nc.vector.scalar_tensor_tensor(out=xa[:, dc, ts(tb, TB)], in0=py,
                                               scalar=b22s[:, dc:dc + 1],
                                               in1=xa[:, dc, ts(tb, TB)],
                                               op0=OP.add, op1=OP.add)
                nc.sync.dma_start(out=yt[dc][:, ts(tb, TB)], in_=xa[:, dc, ts(tb, TB)])


def _prep_weights(norm1_w, mlp1_w, mlp1_b, norm2_w, mlp2_w1, mlp2_b1, mlp2_w2,
                  mlp2_b2):
    W1 = (np.asarray(norm1_w, np.float32)[:, None]
          * np.asarray(mlp1_w, np.float32))
    W21 = (np.asarray(norm2_w, np.float32)[:, None]
           * np.asarray(mlp2_w1, np.float32))
    W22 = np.asarray(mlp2_w2, np.float32)

    w1t = np.ascontiguousarray(
        W1.reshape(CK, P, CK, P).transpose(2, 1, 0, 3).reshape(CK, P, CK * P))
    w21t = np.ascontiguousarray(
        W21.reshape(CK, P, EK, P).transpose(2, 1, 0, 3).reshape(EK, P, CK * P))
    w22t = np.ascontiguousarray(
        W22.reshape(4, 8, P, CK, P).transpose(3, 0, 2, 1, 4).reshape(CK, 4, P, 8 * P))

    b1t = np.ascontiguousarray(np.asarray(mlp1_b, np.float32).reshape(CK, P).T)
    b21t = np.ascontiguousarray(np.asarray(mlp2_b1, np.float32).reshape(EK, P).T)
    b22t = np.ascontiguousarray(np.asarray(mlp2_b2, np.float32).reshape(CK, P).T)

    scaler = np.cumsum(np.arange(1, T + 1, dtype=np.float32), dtype=np.float32)
    sci_b = np.ascontiguousarray(
        np.broadcast_to((1.0 / scaler).astype(np.float32), (P, T)))

    return dict(w1=w1t, b1=b1t, w21=w21t, b21=b21t, w22=w22t, b22=b22t,
                sci=sci_b, onesd=np.ones((P, P), np.float32))


LAST_RESULTS = None


def kernel(x, norm1_w, mlp1_w, mlp1_b, norm2_w, mlp2_w1, mlp2_b1, mlp2_w2,
           mlp2_b2):
    global LAST_RESULTS
    x = np.asarray(x, np.float32)
    assert x.shape == (B, T, C), x.shape

    if "nc" not in _CACHED:
        _CACHED["nc"] = _build_program()
    nc = _CACHED["nc"]

    weights = _prep_weights(norm1_w, mlp1_w, mlp1_b, norm2_w,
                            mlp2_w1, mlp2_b1, mlp2_w2, mlp2_b2)

    in_maps = []
    for b in range(B):
        xt_b = np.ascontiguousarray(x[b].T).reshape(CK, P, T)
        in_maps.append(dict(xt=xt_b, **weights))

    trace = bool(int(os.environ.get("KERNEL_TRACE", "0")))
    res = run_bass_kernel_spmd(nc, in_maps, core_ids=list(range(N_CORES)),
                               trace=trace)
    LAST_RESULTS = res

    y = np.stack([r["yt"].reshape(C, T).T for r in res.results])
    return np.ascontiguousarray(y.astype(np.float32))



# revision 11
# speedup vs baseline: 1.3113x; 1.3113x over previous
"""LinearRNNBlock Trainium2 kernel.

B=8, T=2048, C=1024, EXP=4. Data-parallel over batch: core b computes batch b.

On-chip layout is feature-major [c partitions, t free] end to end:
  - host pre-transposes x[b] -> [C, T] and pre-tiles all weights into
    lhsT blocks, so the device does zero transposes
  - rmsnorm reductions (over C = partitions) go through a ones-matmul on the
    PE whose M=128 output is the per-token sum already broadcast across all
    partitions; rstd is exp(-0.5*ln(ms+eps)) on ScalarE; the squares are one
    3D Square activation per t-block on ScalarE (bf16 out)
  - cumsum over T is a DVE tensor_tensor_scan along the free dim (bf16
    storage, fp32 accumulator), chained across t-blocks via an AP initial;
    the 1/scaler multiply is folded past the mlp1 matmul (linearity)
  - mlp1 runs bf16 (full PE rate at N=512)
  - the 4x MLP runs fp8e4m3 DoubleRow matmuls (K=256/instr, 2x PE rate):
    W21 is host-scaled by 2^12 into e4m3's normal range; W22 is quantized as
    a hi+lo e4m3 pair (both at scale 2^13) so its quantization error drops
    out; inverse scales fold into the ScalarE relu/bias activations
  - relu+bias and the output bias/descale run on ScalarE to keep the DVE
    free for the scan chain and elementwise multiplies
"""

import os
import sys

sys.path.insert(0, "/opt/trn_rl_repo")

from contextlib import ExitStack, nullcontext

import numpy as np
import ml_dtypes

import concourse.bass as bass
import concourse.tile as tile
from concourse import bacc
from concourse import mybir
from concourse.bass import ts
from concourse.bass_utils import run_bass_kernel_spmd

P = 128
B = 8
T = 2048
C = 1024
E = 4096
CK = C // P   # 8 channel chunks
EK = E // P   # 32 expanded chunks
TB = 512      # t-block (max fp32 matmul free dim / one PSUM bank)
NB = T // TB  # 4 t-blocks
EPS = 1e-6
K21 = 2.0 ** 12   # host scale on W21 (|w| <= 1/32 -> <= 128 in e4m3 range)
K22 = 2.0 ** 13   # host scale on W22 (|w| <= 1/64 -> <= 128)

F32 = mybir.dt.float32
F32R = mybir.dt.float32r
BF16 = mybir.dt.bfloat16
F8 = mybir.dt.float8e4
AF = mybir.ActivationFunctionType
OP = mybir.AluOpType
DR = mybir.MatmulPerfMode.DoubleRow

N_CORES = 8

_CACHED = {}


def _build_program(reps=1):
    nc = bacc.Bacc("TRN2", target_bir_lowering=False, debug=False,
                   enable_asserts=False, num_devices=N_CORES)

    xt = nc.dram_tensor("xt", [CK, P, T], F32, kind="ExternalInput").ap()
    w1 = nc.dram_tensor("w1", [CK, P, CK * P], BF16, kind="ExternalInput").ap()
    b1 = nc.dram_tensor("b1", [P, CK], F32, kind="ExternalInput").ap()
    w21 = nc.dram_tensor("w21", [P, EK, 8 * P], F8, kind="ExternalInput").ap()
    b21 = nc.dram_tensor("b21", [P, EK], F32, kind="ExternalInput").ap()
    w22 = nc.dram_tensor("w22", [CK, P, 64 * P], F8, kind="ExternalInput").ap()
    b22 = nc.dram_tensor("b22", [P, CK], F32, kind="ExternalInput").ap()
    sci = nc.dram_tensor("sci", [P, T], F32, kind="ExternalInput").ap()
    onesd = nc.dram_tensor("onesd", [P, P], BF16, kind="ExternalInput").ap()
    yt = nc.dram_tensor("yt", [CK, P, T], F32, kind="ExternalOutput").ap()

    with tile.TileContext(nc) as tc, ExitStack() as ctx:
        consts = ctx.enter_context(tc.tile_pool(name="consts", bufs=1))
        arena = ctx.enter_context(tc.tile_pool(name="arena", bufs=1))
        wp = ctx.enter_context(tc.tile_pool(name="wp", bufs=2))
        bc = ctx.enter_context(tc.tile_pool(name="bc", bufs=1))
        gp = ctx.enter_context(tc.tile_pool(name="gp", bufs=4))
        sqp = ctx.enter_context(tc.tile_pool(name="sqp", bufs=1))
        rp = ctx.enter_context(tc.tile_pool(name="rp", bufs=2))
        ps = ctx.enter_context(tc.tile_pool(name="ps", bufs=1, space="PSUM"))

        # xa: x^T chunks -> out1 (in place) -> y (in place)
        xa = arena.tile([P, CK, T], F32)
        # sa: h1 -> state (raw cumsum, in place); dead after mlp1
        sa = arena.tile([P, CK, T], BF16)
        # w21 stays resident across all t-blocks (4 MB fp8)
        w21r = arena.tile([P, EK, 8 * P], F8)

        epsb = consts.tile([P, 1], F32)
        nc.vector.memset(epsb, EPS)
        onesf = consts.tile([P, P], BF16)
        b1s = consts.tile([P, CK], F32)
        nc.sync.dma_start(out=b1s, in_=b1)
        b21s = consts.tile([P, EK], F32)
        nc.sync.dma_start(out=b21s, in_=b21)
        b22s = consts.tile([P, CK], F32)
        nc.sync.dma_start(out=b22s, in_=b22)
        scib = bc.tile([P, T], F32, tag="sci", name="scib")

        env = dict(locals())
        loop_ctx = tc.For_i(0, reps, 1) if reps > 1 else nullcontext()
        with loop_ctx:
            _emit_body(nc, tc, env)

    nc.compile()
    return nc


def _emit_body(nc, tc, env):
    xa, sa, w21r = env["xa"], env["sa"], env["w21r"]
    onesf, epsb, scib = env["onesf"], env["epsb"], env["scib"]
    b1s, b21s, b22s = env["b1s"], env["b21s"], env["b22s"]
    xt, w1, w21, w22, yt = env["xt"], env["w1"], env["w21"], env["w22"], env["yt"]
    sci, onesd = env["sci"], env["onesd"]
    wp, gp, sqp, rp, ps = env["wp"], env["gp"], env["sqp"], env["rp"], env["ps"]

    # ---- phase A: x load + norm1 + scan, pipelined per t-block ----
    # x chunks first in queue order (they gate everything), then the big
    # non-critical loads (sci, resident w21)
    for tb in range(NB):
        for cc in range(CK):
            nc.sync.dma_start(out=xa[:, cc, ts(tb, TB)],
                              in_=xt[cc][:, ts(tb, TB)])
    nc.sync.dma_start(out=onesf, in_=onesd)
    nc.sync.dma_start(out=scib, in_=sci)
    nc.sync.dma_start(out=w21r, in_=w21)

    rstd1s = []
    for tb in range(NB):
        sq3 = sqp.tile([P, CK, TB], BF16, tag="sq", name="sq3")
        nc.scalar.activation(sq3, xa[:, :, ts(tb, TB)], AF.Square)
        acc = ps.tile([P, TB], F32, tag="acc", bufs=2, name=f"acc1_{tb}")
        for cc in range(CK):
            nc.tensor.matmul(acc, lhsT=onesf, rhs=sq3[:, cc, :],
                             start=(cc == 0), stop=(cc == CK - 1))
        ln1 = gp.tile([P, TB], F32, tag="g", name="ln1")
        nc.scalar.activation(ln1, acc, AF.Ln, bias=epsb, scale=1.0 / C)
        rstd1 = rp.tile([P, TB], F32, tag="r1", name="rstd1")
        nc.scalar.activation(rstd1, ln1, AF.Exp, scale=-0.5)
        rstd1s.append(rstd1)
        for cc in range(CK):
            s_sl = sa[:, cc, ts(tb, TB)]
            nc.vector.tensor_mul(s_sl, xa[:, cc, ts(tb, TB)], rstd1)
            init = 0.0 if tb == 0 else sa[:, cc, tb * TB - 1:tb * TB]
            nc.vector.tensor_tensor_scan(s_sl, s_sl, s_sl, initial=init,
                                         op0=OP.add, op1=OP.bypass)

    # ---- phase B: mlp1 gate; sci (1/scaler) applied post-matmul ----
    for dc in range(CK):
        w1s = wp.tile([P, CK, P], BF16, tag="w1", name="w1s")
        nc.sync.dma_start(out=w1s, in_=w1[dc])
        for tb in range(NB):
            pg = ps.tile([P, TB], F32, tag="mm", bufs=6, name="pg")
            for cc in range(CK):
                nc.tensor.matmul(pg, lhsT=w1s[:, cc, :],
                                 rhs=sa[:, cc, ts(tb, TB)],
                                 start=(cc == 0), stop=(cc == CK - 1))
            zz = gp.tile([P, TB], F32, tag="g", name="zz")
            nc.vector.tensor_mul(zz, pg, scib[:, ts(tb, TB)])
            g = gp.tile([P, TB], F32, tag="g", name="g")
            nc.scalar.activation(g, zz, AF.Sigmoid, bias=b1s[:, dc:dc + 1],
                                 scale=1.0)
            nc.vector.tensor_mul(xa[:, dc, ts(tb, TB)], g,
                                 xa[:, dc, ts(tb, TB)])

    # ---- phase C: norm2 on out1 ----
    rstd2s = []
    for tb in range(NB):
        sq3 = sqp.tile([P, CK, TB], BF16, tag="sq", name="sq3b")
        nc.scalar.activation(sq3, xa[:, :, ts(tb, TB)], AF.Square)
        acc2 = ps.tile([P, TB], F32, tag="acc", bufs=2, name=f"acc2_{tb}")
        for cc in range(CK):
            nc.tensor.matmul(acc2, lhsT=onesf, rhs=sq3[:, cc, :],
                             start=(cc == 0), stop=(cc == CK - 1))
        ln2 = gp.tile([P, TB], F32, tag="g", name="ln2")
        nc.scalar.activation(ln2, acc2, AF.Ln, bias=epsb, scale=1.0 / C)
        rstd2 = rp.tile([P, TB], F32, tag="r2", name="rstd2")
        nc.scalar.activation(rstd2, ln2, AF.Exp, scale=-0.5)
        rstd2s.append(rstd2)

    # ---- phase D: 4x MLP per t-block; fp8 DR up; fp8 DR down with W22
    # quantized hi+lo so only the h2/af casts contribute error ----
    for tb in range(NB):
        h2q = gp.tile([P, CK, TB], F8, tag="h2q", bufs=2, name="h2q")
        for cc in range(CK):
            nc.vector.tensor_mul(h2q[:, cc, :], xa[:, cc, ts(tb, TB)],
                                 rstd2s[tb])
        afq = gp.tile([P, EK, TB], F8, tag="afq", bufs=1, name="afq")
        for ec in range(EK):
            w21v = w21r[:, ec, :].rearrange("p (q j m) -> p q j m", q=4, j=2)
            pa = ps.tile([P, TB], F32, tag="mm", bufs=6, name="pa")
            for p in range(4):
                nc.tensor.matmul(pa, lhsT=w21v[:, p],
                                 rhs=h2q[:, 2 * p:2 * p + 2, :],
                                 start=(p == 0), stop=(p == 3), perf_mode=DR)
            nc.scalar.activation(afq[:, ec, :], pa, AF.Relu,
                                 bias=b21s[:, ec:ec + 1], scale=1.0 / K21)
        for dc in range(CK):
            w22s = wp.tile([P, 32, 2, P], F8, tag="w22", bufs=2, name="w22s")
            nc.sync.dma_start(out=w22s, in_=w22[dc])
            py = ps.tile([P, TB], F32, tag="mm", bufs=6, name="py")
            for p in range(32):
                q = p % 16
                nc.tensor.matmul(py, lhsT=w22s[:, p],
                                 rhs=afq[:, 2 * q:2 * q + 2, :],
                                 start=(p == 0), stop=(p == 31), perf_mode=DR)
            yo = gp.tile([P, TB], F32, tag="g", name="yo")
            nc.scalar.activation(yo, py, AF.Identity, bias=b22s[:, dc:dc + 1],
                                 scale=1.0 / K22)
            nc.vector.tensor_add(xa[:, dc, ts(tb, TB)], yo,
                                 xa[:, dc, ts(tb, TB)])
            nc.sync.dma_start(out=yt[dc][:, ts(tb, TB)],
                              in_=xa[:, dc, ts(tb, TB)])


def _prep_weights(norm1_w, mlp1_w, mlp1_b, norm2_w, mlp2_w1, mlp2_b1, mlp2_w2,
                  mlp2_b2):
    W1 = (np.asarray(norm1_w, np.float32)[:, None]
          * np.asarray(mlp1_w, np.float32))
    W21 = (np.asarray(norm2_w, np.float32)[:, None]
           * np.asarray(mlp2_w1, np.float32))
    W22 = np.asarray(mlp2_w2, np.float32)

    w1t = np.ascontiguousarray(
        W1.reshape(CK, P, CK, P).transpose(2, 1, 0, 3)
        .reshape(CK, P, CK * P)).astype(ml_dtypes.bfloat16)

    # DoubleRow lhsT tiles: [k(partition), chunk, pair, j, m] with K=256 per
    # pair = k-tiles (2p, 2p+1); weights scaled into e4m3 normal range
    w21q = np.ascontiguousarray(
        (W21 * K21).reshape(4, 2, P, EK, P).transpose(2, 3, 0, 1, 4)
        .reshape(P, EK, 8 * P)).astype(ml_dtypes.float8_e4m3)

    # W22: hi+lo e4m3 pair at one scale; pairs 0..15 hi, 16..31 lo
    w22sc = W22 * K22
    w22h = w22sc.astype(ml_dtypes.float8_e4m3)
    w22l = (w22sc - w22h.astype(np.float32)).astype(ml_dtypes.float8_e4m3)

    def dr(w):
        return w.reshape(16, 2, P, CK, P).transpose(3, 2, 0, 1, 4)

    w22q = np.ascontiguousarray(
        np.concatenate([dr(w22h), dr(w22l)], axis=2).reshape(CK, P, 64 * P))

    b1t = np.ascontiguousarray(np.asarray(mlp1_b, np.float32).reshape(CK, P).T)
    b21t = np.ascontiguousarray(np.asarray(mlp2_b1, np.float32).reshape(EK, P).T)
    b22t = np.ascontiguousarray(np.asarray(mlp2_b2, np.float32).reshape(CK, P).T)

    scaler = np.cumsum(np.arange(1, T + 1, dtype=np.float32), dtype=np.float32)
    sci_b = np.ascontiguousarray(
        np.broadcast_to((1.0 / scaler).astype(np.float32), (P, T)))

    return dict(w1=w1t, b1=b1t, w21=w21q, b21=b21t, w22=w22q, b22=b22t,
                sci=sci_b, onesd=np.ones((P, P), ml_dtypes.bfloat16))


LAST_RESULTS = None


def kernel(x, norm1_w, mlp1_w, mlp1_b, norm2_w, mlp2_w1, mlp2_b1, mlp2_w2,
           mlp2_b2):
    global LAST_RESULTS
    x = np.asarray(x, np.float32)
    assert x.shape == (B, T, C), x.shape

    if "nc" not in _CACHED:
        _CACHED["nc"] = _build_program()
    nc = _CACHED["nc"]

    weights = _prep_weights(norm1_w, mlp1_w, mlp1_b, norm2_w,
                            mlp2_w1, mlp2_b1, mlp2_w2, mlp2_b2)

    in_maps = []
    for b in range(B):
        xt_b = np.ascontiguousarray(x[b].T).reshape(CK, P, T)
        in_maps.append(dict(xt=xt_b, **weights))

    trace = bool(int(os.environ.get("KERNEL_TRACE", "0")))
    res = run_bass_kernel_spmd(nc, in_maps, core_ids=list(range(N_CORES)),
                               trace=trace)
    LAST_RESULTS = res

    y = np.stack([r["yt"].reshape(C, T).T for r in res.results])
    return np.ascontiguousarray(y.astype(np.float32))
